# revision 6
# baseline (speedup 1.0000x reference)
"""CrossNetMix (moe_routing) Trainium2 Bass kernel.

Math per layer (B=16384, D=1024, R=64, E=4, L=3):
    g  = softmax(xl @ gates_w.T)                   # [B, E]
    t1 = tanh(einsum('erd,bd->ber', V, xl))        # [B, E, R]
    t2 = tanh(einsum('ers,bes->ber', C, t1))       # [B, E, R]
    d  = einsum('edr,ber->bed', U, t2) + bias      # [B, E, D]
    xl = xl + x0 * sum_e g_e * d_e                 # gated combine + residual

Factorization used on-chip (everything transposed: d on partitions, b on
free dim; batch sharded 8 ways -> B_c = 2048 per core):
  - V matmuls for expert pairs (2R=128 rows) packed into one stationary.
  - C as 128x128 block-diagonal (per pair) single matmul.
  - softmax over E=4 via: gates matmul -> exp (ACT) -> replication
    matmuls (0/1 stationary, K=4) to broadcast exp/sum across the 128
    partitions of the stacked (e, r) layout -> DVE reciprocal + muls.
  - sum_e g_e * (t2_e @ U_e^T) via row-scaling t2 by g then accumulating
    both expert pairs into one PSUM tile.
  - since sum_e g_e = 1, bias is added once (ACT evac with per-partition
    bias), then xl += x0 * (delta + bias) on DVE.
All matmuls run in float32r (TF32-like, ~11-bit mantissa, 1 cyc/row for
N>=256 vs 4 cyc/row for fp32) with fp32 PSUM accumulation.
"""

import numpy as np

import concourse.bass as bass
import concourse.tile as tile
from concourse import bacc, mybir
from concourse.bass_utils import run_bass_kernel_spmd

B, D, R, E, L = 16384, 1024, 64, 4, 3
NCORES = 8
BC = B // NCORES          # 2048 rows per core
NBT = 4                   # b tiles of 512
BT = BC // NBT
NK = D // 128             # 8 k/d tiles

F32R = mybir.dt.float32r
F32 = mybir.dt.float32
AF = mybir.ActivationFunctionType

_cache = {}


def _build(repeat=1, bench=False):
    key = (repeat, bench)
    if key in _cache:
        return _cache[key]
    nc = bacc.Bacc("TRN2", target_bir_lowering=False, debug=False)
    if bench:
        # Timing-only build: no real I/O transfers — all data tensors live
        # in internal DRAM (garbage values; engine timing is data-blind).
        dummy_in = nc.dram_tensor("dummy_in", [1, 1], F32, kind="ExternalInput")
        dummy_out = nc.dram_tensor("dummy_out", [1, 1], F32, kind="ExternalOutput")
        mk = lambda name, shape, dt: nc.dram_tensor(name, shape, dt)
    else:
        mk = lambda name, shape, dt: nc.dram_tensor(name, shape, dt, kind="ExternalInput")
    xT = mk("xT", [D, BC], F32R)
    Vt = mk("Vt", [L, D, 2 * 128], F32R)
    Gt = mk("Gt", [D, E], F32R)
    Cb = mk("Cb", [L, 2, 128, 128], F32R)
    Ut = mk("Ut", [L, 2, 128, D], F32R)
    RP = mk("RP", [3, E, 128], F32R)
    BT_ = mk("BT", [128, L * NK], F32)
    if bench:
        OT = nc.dram_tensor("OT", [D, BC], F32)
    else:
        OT = nc.dram_tensor("OT", [D, BC], F32, kind="ExternalOutput")

    with tile.TileContext(nc) as tc:
        xl = [nc.alloc_sbuf_tensor(f"xl{k}", [128, BC], F32R) for k in range(NK)]
        vt = nc.alloc_sbuf_tensor("vt", [128, L, NK, 256], F32R)
        ut = nc.alloc_sbuf_tensor("ut", [128, L, 2, D], F32R)
        cb = nc.alloc_sbuf_tensor("cb", [128, L, 2, 128], F32R)
        gt = nc.alloc_sbuf_tensor("gt", [128, NK, E], F32R)
        rp = nc.alloc_sbuf_tensor("rp", [E, 3, 128], F32R)
        bt_sb = nc.alloc_sbuf_tensor("bt_sb", [128, L * NK], F32)

        # --- param + input loads (order matters only as a scheduler hint) ---
        nc.sync.dma_start(gt.ap(), Gt.ap().rearrange("(k p) m -> p k m", p=128))
        nc.sync.dma_start(rp.ap(), RP.ap().rearrange("j e m -> e j m"))
        nc.sync.dma_start(bt_sb.ap(), BT_.ap())
        for l in range(L):
            nc.sync.dma_start(
                vt.ap()[:, l], Vt.ap()[l].rearrange("(k p) m -> p k m", p=128)
            )
            for p in range(2):
                nc.sync.dma_start(cb.ap()[:, l, p], Cb.ap()[l, p])
        # xl quadrants, bt-major so bt=0 compute starts early
        for bt in range(NBT):
            for k in range(NK):
                nc.sync.dma_start(
                    xl[k].ap()[:, bass.ts(bt, BT)],
                    xT.ap()[bass.ts(k, 128), bass.ts(bt, BT)],
                )
        for l in range(L):
            for p in range(2):
                nc.sync.dma_start(ut.ap()[:, l, p], Ut.ap()[l, p])

        from contextlib import ExitStack

        ctx = ExitStack()
        ps_mm = ctx.enter_context(tc.tile_pool(name="ps_mm", bufs=3, space="PSUM"))
        ps_g = ctx.enter_context(tc.tile_pool(name="ps_g", bufs=2, space="PSUM"))
        ps_s = ctx.enter_context(tc.tile_pool(name="ps_s", bufs=1, space="PSUM"))
        ps_e = ctx.enter_context(tc.tile_pool(name="ps_e", bufs=2, space="PSUM"))
        sb_t1 = ctx.enter_context(tc.tile_pool(name="sb_t1", bufs=3))
        sb_t2 = ctx.enter_context(tc.tile_pool(name="sb_t2", bufs=3))
        sb_t2s = ctx.enter_context(tc.tile_pool(name="sb_t2s", bufs=2))
        sb_rs = ctx.enter_context(tc.tile_pool(name="sb_rs", bufs=2))
        sb_e4 = ctx.enter_context(tc.tile_pool(name="sb_e4", bufs=2))
        sb_tm = ctx.enter_context(tc.tile_pool(name="sb_tm", bufs=4))
        sb_x0 = ctx.enter_context(tc.tile_pool(name="sb_x0", bufs=4))
        sb_ot = ctx.enter_context(tc.tile_pool(name="sb_ot", bufs=3))

        def body(_iv=None):
            for l in range(L):
                for bt in range(NBT):
                    bs = bass.ts(bt, BT)
                    # ---- gates logits + exp ----
                    gps = ps_g.tile([E, BT], F32, tag="g")
                    for k in range(NK):
                        nc.tensor.matmul(
                            gps[:], gt.ap()[:, k], xl[k].ap()[:, bs],
                            start=(k == 0), stop=(k == NK - 1),
                        )
                    e4 = sb_e4.tile([E, BT], F32R, tag="e4")
                    nc.scalar.activation(e4[:], gps[:], AF.Exp)
                    # ---- sum over experts, replicated to 128 partitions ----
                    sps = ps_s.tile([128, BT], F32, tag="s")
                    nc.tensor.matmul(sps[:], rp.ap()[:, 2], e4[:], start=True, stop=True)
                    rs = sb_rs.tile([128, BT], F32, tag="rs")
                    nc.vector.reciprocal(rs[:], sps[:])

                    t2s = []
                    for p in range(2):
                        # ---- V pass (expert pair stationary) + tanh ----
                        vps = ps_mm.tile([128, BT], F32, tag="mm")
                        for k in range(NK):
                            nc.tensor.matmul(
                                vps[:], vt.ap()[:, l, k, bass.ts(p, 128)],
                                xl[k].ap()[:, bs],
                                start=(k == 0), stop=(k == NK - 1),
                            )
                        t1 = sb_t1.tile([128, BT], F32R, tag="t1")
                        nc.scalar.activation(t1[:], vps[:], AF.Tanh)
                        # ---- C pass (block-diagonal) + tanh ----
                        cps = ps_mm.tile([128, BT], F32, tag="mm")
                        nc.tensor.matmul(cps[:], cb.ap()[:, l, p], t1[:], start=True, stop=True)
                        t2 = sb_t2.tile([128, BT], F32, tag="t2")
                        nc.scalar.activation(t2[:], cps[:], AF.Tanh)
                        # ---- gate scaling: t2 * exp_rep * recip(sum_rep) ----
                        eps_ = ps_e.tile([128, BT], F32, tag="e")
                        nc.tensor.matmul(eps_[:], rp.ap()[:, p], e4[:], start=True, stop=True)
                        tm = sb_tm.tile([128, BT], F32, tag="tm")
                        nc.vector.tensor_mul(tm[:], t2[:], eps_[:])
                        t2sp = sb_t2s.tile([128, BT], F32R, tag=f"t2s{p}")
                        nc.vector.tensor_mul(t2sp[:], tm[:], rs[:])
                        t2s.append(t2sp)
                    # ---- U pass + epilogue ----
                    for dt in range(NK):
                        ups = ps_mm.tile([128, BT], F32, tag="mm")
                        nc.tensor.matmul(
                            ups[:], ut.ap()[:, l, 0, bass.ts(dt, 128)], t2s[0][:],
                            start=True, stop=False,
                        )
                        nc.tensor.matmul(
                            ups[:], ut.ap()[:, l, 1, bass.ts(dt, 128)], t2s[1][:],
                            start=False, stop=True,
                        )
                        # delta + bias (per-partition) -> SBUF via ACT
                        tm2 = sb_tm.tile([128, BT], F32, tag="tm")
                        nc.scalar.activation(
                            tm2[:], ups[:], AF.Identity,
                            bias=bt_sb.ap()[:, l * NK + dt : l * NK + dt + 1],
                        )
                        if l == 0:
                            x0ap = xl[dt].ap()[:, bs].bitcast(F32)
                        else:
                            x0t = sb_x0.tile([128, BT], F32R, tag="x0")
                            nc.sync.dma_start(
                                x0t[:], xT.ap()[bass.ts(dt, 128), bs]
                            )
                            x0ap = x0t[:].bitcast(F32)
                        tm3 = sb_tm.tile([128, BT], F32, tag="tm")
                        nc.vector.tensor_mul(tm3[:], tm2[:], x0ap)
                        if l < L - 1:
                            nc.vector.tensor_add(
                                xl[dt].ap()[:, bs],
                                xl[dt].ap()[:, bs].bitcast(F32),
                                tm3[:],
                            )
                        else:
                            ot = sb_ot.tile([128, BT], F32, tag="ot")
                            nc.vector.tensor_add(
                                ot[:], xl[dt].ap()[:, bs].bitcast(F32), tm3[:]
                            )
                            nc.sync.dma_start(
                                OT.ap()[bass.ts(dt, 128), bs], ot[:]
                            )

        if repeat == 1:
            body()
        else:
            with tc.For_i(0, repeat, 1) as _i:
                body(_i)
        if bench:
            dtile = sb_tm.tile([1, 1], F32, tag="dummy")
            nc.sync.dma_start(dtile[:], dummy_in.ap())
            nc.sync.dma_start(dummy_out.ap(), dtile[:])
        ctx.close()

    nc.compile()
    _cache[key] = nc
    return nc


def _prep(x, U, V, C, bias, gates_w):
    """Host-side layout prep. Returns (shared dict, list of per-core dicts)."""
    x = np.ascontiguousarray(x, dtype=np.float32)
    Vt = np.ascontiguousarray(
        V.astype(np.float32).transpose(0, 3, 1, 2).reshape(L, D, E * R)
    )
    Gt = np.ascontiguousarray(gates_w.astype(np.float32).T)
    Cbd = np.zeros((L, 2, 128, 128), dtype=np.float32)
    for l in range(L):
        for p in range(2):
            Cbd[l, p, :R, :R] = C[l, 2 * p].T
            Cbd[l, p, R:, R:] = C[l, 2 * p + 1].T
    Ut = np.zeros((L, 2, 128, D), dtype=np.float32)
    for l in range(L):
        for p in range(2):
            Ut[l, p, :R] = U[l, 2 * p].T
            Ut[l, p, R:] = U[l, 2 * p + 1].T
    RP = np.zeros((3, E, 128), dtype=np.float32)
    for p in range(2):
        for e in range(E):
            if e // 2 == p:
                RP[p, e, (e % 2) * R : (e % 2 + 1) * R] = 1.0
    RP[2] = 1.0
    BTm = np.ascontiguousarray(
        bias.astype(np.float32).reshape(L, NK, 128).transpose(2, 0, 1).reshape(128, L * NK)
    )
    shared = {"Vt": Vt, "Gt": Gt, "Cb": Cbd, "Ut": Ut, "RP": RP, "BT": BTm}
    per_core = []
    for i in range(NCORES):
        xTi = np.ascontiguousarray(x[i * BC : (i + 1) * BC].T)
        per_core.append({"xT": xTi, **shared})
    return per_core


def kernel(x, U, V, C, bias, gates_w):
    nc = _build(1)
    in_maps = _prep(x, U, V, C, bias, gates_w)
    res = run_bass_kernel_spmd(nc, in_maps, list(range(NCORES)))
    out = np.empty((B, D), dtype=np.float32)
    for i in range(NCORES):
        out[i * BC : (i + 1) * BC] = res.results[i]["OT"].T
    return out


if __name__ == "__main__":
    rng = np.random.default_rng(0)
    x = rng.standard_normal((B, D), dtype=np.float32)
    su = (2.0 / (D + R)) ** 0.5
    sc = (2.0 / (R + R)) ** 0.5
    U_ = rng.standard_normal((L, E, D, R), dtype=np.float32) * su
    V_ = rng.standard_normal((L, E, R, D), dtype=np.float32) * su
    C_ = rng.standard_normal((L, E, R, R), dtype=np.float32) * sc
    b_ = np.zeros((L, D), dtype=np.float32)
    g_ = rng.standard_normal((E, D), dtype=np.float32) / np.sqrt(D)
    out = kernel(x, U_, V_, C_, b_, g_)

    # numpy reference
    x0, xl = x, x.astype(np.float64)
    for i in range(L):
        logits = xl @ g_.T.astype(np.float64)
        ex = np.exp(logits - logits.max(axis=1, keepdims=True))
        g = ex / ex.sum(axis=1, keepdims=True)
        t = np.tanh(np.einsum("erd,bd->ber", V_[i].astype(np.float64), xl))
        t = np.tanh(np.einsum("ers,bes->ber", C_[i].astype(np.float64), t))
        t = np.einsum("edr,ber->bed", U_[i].astype(np.float64), t) + b_[i][None, None, :]
        t = x0[:, None, :] * t
        xl = np.einsum("bed,be->bd", t, g) + xl
    err = np.abs(out - xl)
    print(f"absmax={err.max():.4e} rel={err.max()/np.abs(xl).max():.4e}")


# revision 12
# speedup vs baseline: 1.1577x; 1.1577x over previous
"""CrossNetMix (moe_routing) Trainium2 Bass kernel.

Math per layer (B=16384, D=1024, R=64, E=4, L=3):
    g  = softmax(xl @ gates_w.T)                   # [B, E]
    t1 = tanh(einsum('erd,bd->ber', V, xl))        # [B, E, R]
    t2 = tanh(einsum('ers,bes->ber', C, t1))       # [B, E, R]
    d  = einsum('edr,ber->bed', U, t2) + bias      # [B, E, D]
    xl = xl + x0 * sum_e g_e * d_e                 # gated combine + residual

Factorization used on-chip (everything transposed: d on partitions, b on
free dim; batch sharded 8 ways -> B_c = 2048 per core):
  - V matmuls for expert pairs (2R=128 rows) packed into one stationary.
  - C as 128x128 block-diagonal (per pair) single matmul.
  - softmax over E=4 via: gates matmul -> exp (ACT) -> replication
    matmuls (0/1 stationary, K=4) to broadcast exp/sum across the 128
    partitions of the stacked (e, r) layout -> DVE reciprocal + muls.
  - sum_e g_e * (t2_e @ U_e^T) via row-scaling t2 by g then accumulating
    both expert pairs into one PSUM tile.
  - since sum_e g_e = 1, bias is added once (ACT evac with per-partition
    bias), then xl += x0 * (delta + bias) on DVE.
All matmuls run in float32r (TF32-like, ~11-bit mantissa, 1 cyc/row for
N>=256 vs 4 cyc/row for fp32) with fp32 PSUM accumulation.
"""

import numpy as np

import concourse.bass as bass
import concourse.tile as tile
from concourse import bacc, mybir
from concourse.bass_utils import run_bass_kernel_spmd

B, D, R, E, L = 16384, 1024, 64, 4, 3
NCORES = 8
BC = B // NCORES          # 2048 rows per core
NBT = 4                   # b tiles of 512
BT = BC // NBT
NK = D // 128             # 8 k/d tiles

F32R = mybir.dt.float32r
F32 = mybir.dt.float32
AF = mybir.ActivationFunctionType

_cache = {}
ADD_ENGINE = "dve"  # "dve" or "pool" — engine for the xl-residual add


def _build(repeat=1, bench=False):
    key = (repeat, bench)
    if key in _cache:
        return _cache[key]
    nc = bacc.Bacc("TRN2", target_bir_lowering=False, debug=False)
    if bench:
        # Timing-only build: no real I/O transfers — all data tensors live
        # in internal DRAM (garbage values; engine timing is data-blind).
        dummy_in = nc.dram_tensor("dummy_in", [1, 1], F32, kind="ExternalInput")
        dummy_out = nc.dram_tensor("dummy_out", [1, 1], F32, kind="ExternalOutput")
        mk = lambda name, shape, dt: nc.dram_tensor(name, shape, dt)
    else:
        mk = lambda name, shape, dt: nc.dram_tensor(name, shape, dt, kind="ExternalInput")
    xT = mk("xT", [D, BC], F32R)
    Vt = mk("Vt", [L, D, 2 * 128], F32R)
    Gt = mk("Gt", [D, E], F32R)
    Cb = mk("Cb", [L, 2, 128, 128], F32R)
    Ut = mk("Ut", [L, 2, 128, D], F32R)
    RP = mk("RP", [3, E, 128], F32R)
    BT_ = mk("BT", [128, L * NK], F32)
    if bench:
        OT = nc.dram_tensor("OT", [D, BC], F32)
    else:
        OT = nc.dram_tensor("OT", [D, BC], F32, kind="ExternalOutput")

    with tile.TileContext(nc) as tc:
        xl = [nc.alloc_sbuf_tensor(f"xl{k}", [128, BC], F32R) for k in range(NK)]
        vt = nc.alloc_sbuf_tensor("vt", [128, L, NK, 256], F32R)
        ut = nc.alloc_sbuf_tensor("ut", [128, L, 2, D], F32R)
        cb = nc.alloc_sbuf_tensor("cb", [128, L, 2, 128], F32R)
        gt = nc.alloc_sbuf_tensor("gt", [128, NK, E], F32R)
        rp = nc.alloc_sbuf_tensor("rp", [E, 3, 128], F32R)
        bt_sb = nc.alloc_sbuf_tensor("bt_sb", [128, L * NK], F32)

        # --- param + input loads (order matters only as a scheduler hint) ---
        nc.sync.dma_start(gt.ap(), Gt.ap().rearrange("(k p) m -> p k m", p=128))
        nc.sync.dma_start(rp.ap(), RP.ap().rearrange("j e m -> e j m"))
        nc.sync.dma_start(bt_sb.ap(), BT_.ap())
        for l in range(L):
            nc.sync.dma_start(
                vt.ap()[:, l], Vt.ap()[l].rearrange("(k p) m -> p k m", p=128)
            )
            for p in range(2):
                nc.sync.dma_start(cb.ap()[:, l, p], Cb.ap()[l, p])
        # xl quadrants, bt-major so bt=0 compute starts early
        for bt in range(NBT):
            for k in range(NK):
                nc.sync.dma_start(
                    xl[k].ap()[:, bass.ts(bt, BT)],
                    xT.ap()[bass.ts(k, 128), bass.ts(bt, BT)],
                )
        for l in range(L):
            for p in range(2):
                nc.sync.dma_start(ut.ap()[:, l, p], Ut.ap()[l, p])

        from contextlib import ExitStack

        ctx = ExitStack()
        # PSUM budget is 8 banks ([128,512]f32 = 1 bank). Separate pools for
        # the V/C chain and the U chain so U-pass slot pressure cannot stall
        # the next b-tile's V accumulation.
        ps_vc = ctx.enter_context(tc.tile_pool(name="ps_vc", bufs=2, space="PSUM"))
        ps_u = ctx.enter_context(tc.tile_pool(name="ps_u", bufs=2, space="PSUM"))
        ps_g = ctx.enter_context(tc.tile_pool(name="ps_g", bufs=1, space="PSUM"))
        ps_s = ctx.enter_context(tc.tile_pool(name="ps_s", bufs=1, space="PSUM"))
        ps_e = ctx.enter_context(tc.tile_pool(name="ps_e", bufs=2, space="PSUM"))
        sb_t1 = ctx.enter_context(tc.tile_pool(name="sb_t1", bufs=4))
        sb_t2 = ctx.enter_context(tc.tile_pool(name="sb_t2", bufs=4))
        sb_t2s = ctx.enter_context(tc.tile_pool(name="sb_t2s", bufs=3))
        sb_rs = ctx.enter_context(tc.tile_pool(name="sb_rs", bufs=2))
        sb_e4 = ctx.enter_context(tc.tile_pool(name="sb_e4", bufs=2))
        sb_tm = ctx.enter_context(tc.tile_pool(name="sb_tm", bufs=6))
        sb_x0 = ctx.enter_context(tc.tile_pool(name="sb_x0", bufs=6))
        sb_ot = ctx.enter_context(tc.tile_pool(name="sb_ot", bufs=4))

        def body(_iv=None):
            for l in range(L):
                for bt in range(NBT):
                    bs = bass.ts(bt, BT)
                    # ---- gates logits + exp ----
                    gps = ps_g.tile([E, BT], F32, tag="g")
                    for k in range(NK):
                        nc.tensor.matmul(
                            gps[:], gt.ap()[:, k], xl[k].ap()[:, bs],
                            start=(k == 0), stop=(k == NK - 1),
                        )
                    e4 = sb_e4.tile([E, BT], F32R, tag="e4")
                    nc.scalar.activation(e4[:], gps[:], AF.Exp)
                    # ---- softmax normalize in [4, b] space: g4 = e4 / sum ----
                    sps = ps_s.tile([E, BT], F32, tag="s")
                    nc.tensor.matmul(sps[:], rp.ap()[:, 2, :E], e4[:], start=True, stop=True)
                    rs = sb_rs.tile([E, BT], F32, tag="rs")
                    nc.vector.reciprocal(rs[:], sps[:])
                    g4 = sb_e4.tile([E, BT], F32R, tag="g4")
                    nc.vector.tensor_mul(g4[:], e4[:].bitcast(F32), rs[:])

                    t2s = []
                    for p in range(2):
                        # ---- V pass (expert pair stationary) + tanh ----
                        vps = ps_vc.tile([128, BT], F32, tag="vc")
                        for k in range(NK):
                            nc.tensor.matmul(
                                vps[:], vt.ap()[:, l, k, bass.ts(p, 128)],
                                xl[k].ap()[:, bs],
                                start=(k == 0), stop=(k == NK - 1),
                            )
                        t1 = sb_t1.tile([128, BT], F32R, tag="t1")
                        nc.scalar.activation(t1[:], vps[:], AF.Tanh)
                        # ---- C pass (block-diagonal) + tanh ----
                        cps = ps_vc.tile([128, BT], F32, tag="vc")
                        nc.tensor.matmul(cps[:], cb.ap()[:, l, p], t1[:], start=True, stop=True)
                        t2 = sb_t2.tile([128, BT], F32, tag="t2")
                        nc.scalar.activation(t2[:], cps[:], AF.Tanh)
                        # ---- gate scaling: t2 * g_rep (g replicated via PE) ----
                        eps_ = ps_e.tile([128, BT], F32, tag="e")
                        nc.tensor.matmul(eps_[:], rp.ap()[:, p], g4[:], start=True, stop=True)
                        t2sp = sb_t2s.tile([128, BT], F32R, tag=f"t2s{p}")
                        nc.vector.tensor_mul(t2sp[:], t2[:], eps_[:])
                        t2s.append(t2sp)
                    # ---- U pass + epilogue ----
                    for dt in range(NK):
                        ups = ps_u.tile([128, BT], F32, tag="u")
                        nc.tensor.matmul(
                            ups[:], ut.ap()[:, l, 0, bass.ts(dt, 128)], t2s[0][:],
                            start=True, stop=False,
                        )
                        nc.tensor.matmul(
                            ups[:], ut.ap()[:, l, 1, bass.ts(dt, 128)], t2s[1][:],
                            start=False, stop=True,
                        )
                        # delta + bias (per-partition) -> SBUF via ACT
                        tm2 = sb_tm.tile([128, BT], F32, tag="tm")
                        nc.scalar.activation(
                            tm2[:], ups[:], AF.Identity,
                            bias=bt_sb.ap()[:, l * NK + dt : l * NK + dt + 1],
                        )
                        if l == 0:
                            x0ap = xl[dt].ap()[:, bs].bitcast(F32)
                        else:
                            x0t = sb_x0.tile([128, BT], F32R, tag="x0")
                            nc.sync.dma_start(
                                x0t[:], xT.ap()[bass.ts(dt, 128), bs]
                            )
                            x0ap = x0t[:].bitcast(F32)
                        tm3 = sb_tm.tile([128, BT], F32, tag="tm")
                        nc.vector.tensor_mul(tm3[:], tm2[:], x0ap)
                        adder = nc.gpsimd if ADD_ENGINE == "pool" else nc.vector
                        if l < L - 1:
                            adder.tensor_add(
                                xl[dt].ap()[:, bs],
                                xl[dt].ap()[:, bs].bitcast(F32),
                                tm3[:],
                            )
                        else:
                            ot = sb_ot.tile([128, BT], F32, tag="ot")
                            adder.tensor_add(
                                ot[:], xl[dt].ap()[:, bs].bitcast(F32), tm3[:]
                            )
                            nc.sync.dma_start(
                                OT.ap()[bass.ts(dt, 128), bs], ot[:]
                            )

        if repeat == 1:
            body()
        else:
            with tc.For_i(0, repeat, 1) as _i:
                body(_i)
        if bench:
            dtile = sb_tm.tile([1, 1], F32, tag="dummy")
            nc.sync.dma_start(dtile[:], dummy_in.ap())
            nc.sync.dma_start(dummy_out.ap(), dtile[:])
        ctx.close()

    nc.compile()
    _cache[key] = nc
    return nc


def _prep(x, U, V, C, bias, gates_w):
    """Host-side layout prep. Returns (shared dict, list of per-core dicts)."""
    x = np.ascontiguousarray(x, dtype=np.float32)
    Vt = np.ascontiguousarray(
        V.astype(np.float32).transpose(0, 3, 1, 2).reshape(L, D, E * R)
    )
    Gt = np.ascontiguousarray(gates_w.astype(np.float32).T)
    Cbd = np.zeros((L, 2, 128, 128), dtype=np.float32)
    for l in range(L):
        for p in range(2):
            Cbd[l, p, :R, :R] = C[l, 2 * p].T
            Cbd[l, p, R:, R:] = C[l, 2 * p + 1].T
    Ut = np.zeros((L, 2, 128, D), dtype=np.float32)
    for l in range(L):
        for p in range(2):
            Ut[l, p, :R] = U[l, 2 * p].T
            Ut[l, p, R:] = U[l, 2 * p + 1].T
    RP = np.zeros((3, E, 128), dtype=np.float32)
    for p in range(2):
        for e in range(E):
            if e // 2 == p:
                RP[p, e, (e % 2) * R : (e % 2 + 1) * R] = 1.0
    RP[2] = 1.0
    BTm = np.ascontiguousarray(
        bias.astype(np.float32).reshape(L, NK, 128).transpose(2, 0, 1).reshape(128, L * NK)
    )
    shared = {"Vt": Vt, "Gt": Gt, "Cb": Cbd, "Ut": Ut, "RP": RP, "BT": BTm}
    per_core = []
    for i in range(NCORES):
        xTi = np.ascontiguousarray(x[i * BC : (i + 1) * BC].T)
        per_core.append({"xT": xTi, **shared})
    return per_core


def kernel(x, U, V, C, bias, gates_w):
    nc = _build(1)
    in_maps = _prep(x, U, V, C, bias, gates_w)
    res = run_bass_kernel_spmd(nc, in_maps, list(range(NCORES)))
    out = np.empty((B, D), dtype=np.float32)
    for i in range(NCORES):
        out[i * BC : (i + 1) * BC] = res.results[i]["OT"].T
    return out


if __name__ == "__main__":
    rng = np.random.default_rng(0)
    x = rng.standard_normal((B, D), dtype=np.float32)
    su = (2.0 / (D + R)) ** 0.5
    sc = (2.0 / (R + R)) ** 0.5
    U_ = rng.standard_normal((L, E, D, R), dtype=np.float32) * su
    V_ = rng.standard_normal((L, E, R, D), dtype=np.float32) * su
    C_ = rng.standard_normal((L, E, R, R), dtype=np.float32) * sc
    b_ = np.zeros((L, D), dtype=np.float32)
    g_ = rng.standard_normal((E, D), dtype=np.float32) / np.sqrt(D)
    out = kernel(x, U_, V_, C_, b_, g_)

    # numpy reference
    x0, xl = x, x.astype(np.float64)
    for i in range(L):
        logits = xl @ g_.T.astype(np.float64)
        ex = np.exp(logits - logits.max(axis=1, keepdims=True))
        g = ex / ex.sum(axis=1, keepdims=True)
        t = np.tanh(np.einsum("erd,bd->ber", V_[i].astype(np.float64), xl))
        t = np.tanh(np.einsum("ers,bes->ber", C_[i].astype(np.float64), t))
        t = np.einsum("edr,ber->bed", U_[i].astype(np.float64), t) + b_[i][None, None, :]
        t = x0[:, None, :] * t
        xl = np.einsum("bed,be->bd", t, g) + xl
    err = np.abs(out - xl)
    print(f"absmax={err.max():.4e} rel={err.max()/np.abs(xl).max():.4e}")


# revision 13
# speedup vs baseline: 1.3090x; 1.1307x over previous
"""CrossNetMix (moe_routing) Trainium2 Bass kernel.

Math per layer (B=16384, D=1024, R=64, E=4, L=3):
    g  = softmax(xl @ gates_w.T)                   # [B, E]
    t1 = tanh(einsum('erd,bd->ber', V, xl))        # [B, E, R]
    t2 = tanh(einsum('ers,bes->ber', C, t1))       # [B, E, R]
    d  = einsum('edr,ber->bed', U, t2) + bias      # [B, E, D]
    xl = xl + x0 * sum_e g_e * d_e                 # gated combine + residual

Factorization used on-chip (everything transposed: d on partitions, b on
free dim; batch sharded 8 ways -> B_c = 2048 per core):
  - V matmuls for expert pairs (2R=128 rows) packed into one stationary.
  - C as 128x128 block-diagonal (per pair) single matmul.
  - softmax over E=4 via: gates matmul -> exp (ACT) -> replication
    matmuls (0/1 stationary, K=4) to broadcast exp/sum across the 128
    partitions of the stacked (e, r) layout -> DVE reciprocal + muls.
  - sum_e g_e * (t2_e @ U_e^T) via row-scaling t2 by g then accumulating
    both expert pairs into one PSUM tile.
  - since sum_e g_e = 1, bias is added once (ACT evac with per-partition
    bias), then xl += x0 * (delta + bias) on DVE.
All matmuls run in float32r (TF32-like, ~11-bit mantissa, 1 cyc/row for
N>=256 vs 4 cyc/row for fp32) with fp32 PSUM accumulation.
"""

import numpy as np

import concourse.bass as bass
import concourse.tile as tile
from concourse import bacc, mybir
from concourse.bass_utils import run_bass_kernel_spmd

B, D, R, E, L = 16384, 1024, 64, 4, 3
NCORES = 8
BC = B // NCORES          # 2048 rows per core
NBT = 4                   # b tiles of 512
BT = BC // NBT
NK = D // 128             # 8 k/d tiles

F32R = mybir.dt.float32r
F32 = mybir.dt.float32
AF = mybir.ActivationFunctionType

_cache = {}
ADD_ENGINE = "dve"  # "dve" or "pool" — engine for the xl-residual add


def _build(repeat=1, bench=False):
    key = (repeat, bench)
    if key in _cache:
        return _cache[key]
    nc = bacc.Bacc("TRN2", target_bir_lowering=False, debug=False)
    if bench:
        # Timing-only build: no real I/O transfers — all data tensors live
        # in internal DRAM (garbage values; engine timing is data-blind).
        dummy_in = nc.dram_tensor("dummy_in", [1, 1], F32, kind="ExternalInput")
        dummy_out = nc.dram_tensor("dummy_out", [1, 1], F32, kind="ExternalOutput")
        mk = lambda name, shape, dt: nc.dram_tensor(name, shape, dt)
    else:
        mk = lambda name, shape, dt: nc.dram_tensor(name, shape, dt, kind="ExternalInput")
    xT = mk("xT", [D, BC], F32R)
    Vt = mk("Vt", [L, D, 2 * 128], F32R)
    Gt = mk("Gt", [D, E], F32R)
    Cb = mk("Cb", [L, 2, 128, 128], F32R)
    Ut = mk("Ut", [L, 2, 128, D], F32R)
    RP = mk("RP", [3, E, 128], F32R)
    BT_ = mk("BT", [128, L * NK], F32)
    if bench:
        OT = nc.dram_tensor("OT", [D, BC], F32)
    else:
        OT = nc.dram_tensor("OT", [D, BC], F32, kind="ExternalOutput")

    with tile.TileContext(nc) as tc:
        xl = [nc.alloc_sbuf_tensor(f"xl{k}", [128, BC], F32R) for k in range(NK)]
        vt = nc.alloc_sbuf_tensor("vt", [128, L, NK, 256], F32R)
        ut = nc.alloc_sbuf_tensor("ut", [128, L, 2, D], F32R)
        cb = nc.alloc_sbuf_tensor("cb", [128, L, 2, 128], F32R)
        gt = nc.alloc_sbuf_tensor("gt", [128, NK, E], F32R)
        rp = nc.alloc_sbuf_tensor("rp", [E, 3, 128], F32R)
        bt_sb = nc.alloc_sbuf_tensor("bt_sb", [128, L * NK], F32)

        # --- param + input loads (order matters only as a scheduler hint) ---
        nc.sync.dma_start(gt.ap(), Gt.ap().rearrange("(k p) m -> p k m", p=128))
        nc.sync.dma_start(rp.ap(), RP.ap().rearrange("j e m -> e j m"))
        nc.sync.dma_start(bt_sb.ap(), BT_.ap())
        for l in range(L):
            nc.sync.dma_start(
                vt.ap()[:, l], Vt.ap()[l].rearrange("(k p) m -> p k m", p=128)
            )
            for p in range(2):
                nc.sync.dma_start(cb.ap()[:, l, p], Cb.ap()[l, p])
        # xl quadrants, bt-major so bt=0 compute starts early
        for bt in range(NBT):
            for k in range(NK):
                nc.sync.dma_start(
                    xl[k].ap()[:, bass.ts(bt, BT)],
                    xT.ap()[bass.ts(k, 128), bass.ts(bt, BT)],
                )
        for l in range(L):
            for p in range(2):
                nc.sync.dma_start(ut.ap()[:, l, p], Ut.ap()[l, p])

        from contextlib import ExitStack

        ctx = ExitStack()
        # PSUM budget is 8 banks ([128,512]f32 = 1 bank). Separate pools for
        # the V/C chain and the U chain so U-pass slot pressure cannot stall
        # the next b-tile's V accumulation.
        ps_vc = ctx.enter_context(tc.tile_pool(name="ps_vc", bufs=2, space="PSUM"))
        ps_u = ctx.enter_context(tc.tile_pool(name="ps_u", bufs=2, space="PSUM"))
        ps_gs = ctx.enter_context(tc.tile_pool(name="ps_gs", bufs=2, space="PSUM"))
        ps_e = ctx.enter_context(tc.tile_pool(name="ps_e", bufs=2, space="PSUM"))
        sb_t1 = ctx.enter_context(tc.tile_pool(name="sb_t1", bufs=4))
        sb_t2 = ctx.enter_context(tc.tile_pool(name="sb_t2", bufs=4))
        sb_t2s = ctx.enter_context(tc.tile_pool(name="sb_t2s", bufs=3))
        sb_rs = ctx.enter_context(tc.tile_pool(name="sb_rs", bufs=2))
        sb_e4 = ctx.enter_context(tc.tile_pool(name="sb_e4", bufs=2))
        sb_tm = ctx.enter_context(tc.tile_pool(name="sb_tm", bufs=6))
        sb_x0 = ctx.enter_context(tc.tile_pool(name="sb_x0", bufs=6))
        sb_ot = ctx.enter_context(tc.tile_pool(name="sb_ot", bufs=4))

        def body(_iv=None):
            for l in range(L):
                for bt in range(NBT):
                    bs = bass.ts(bt, BT)
                    # ---- gates logits + exp ----
                    gps = ps_gs.tile([E, BT], F32, tag="gs")
                    for k in range(NK):
                        nc.tensor.matmul(
                            gps[:], gt.ap()[:, k], xl[k].ap()[:, bs],
                            start=(k == 0), stop=(k == NK - 1),
                        )
                    e4 = sb_e4.tile([E, BT], F32R, tag="e4")
                    nc.scalar.activation(e4[:], gps[:], AF.Exp)
                    # ---- softmax normalize in [4, b] space: g4 = e4 / sum ----
                    sps = ps_gs.tile([E, BT], F32, tag="gs")
                    nc.tensor.matmul(sps[:], rp.ap()[:, 2, :E], e4[:], start=True, stop=True)
                    rs = sb_rs.tile([E, BT], F32, tag="rs")
                    nc.vector.reciprocal_approx_fast(rs[:], sps[:])
                    g4 = sb_e4.tile([E, BT], F32R, tag="g4")
                    nc.vector.tensor_mul(g4[:], e4[:].bitcast(F32), rs[:])

                    t2s = []
                    for p in range(2):
                        # ---- V pass (expert pair stationary) + tanh ----
                        vps = ps_vc.tile([128, BT], F32, tag="vc")
                        for k in range(NK):
                            nc.tensor.matmul(
                                vps[:], vt.ap()[:, l, k, bass.ts(p, 128)],
                                xl[k].ap()[:, bs],
                                start=(k == 0), stop=(k == NK - 1),
                            )
                        t1 = sb_t1.tile([128, BT], F32R, tag="t1")
                        nc.scalar.activation(t1[:], vps[:], AF.Tanh)
                        # ---- C pass (block-diagonal) + tanh ----
                        cps = ps_vc.tile([128, BT], F32, tag="vc")
                        nc.tensor.matmul(cps[:], cb.ap()[:, l, p], t1[:], start=True, stop=True)
                        t2 = sb_t2.tile([128, BT], F32, tag="t2")
                        nc.scalar.activation(t2[:], cps[:], AF.Tanh)
                        # ---- gate scaling: t2 * g_rep (g replicated via PE) ----
                        eps_ = ps_e.tile([128, BT], F32, tag="e")
                        nc.tensor.matmul(eps_[:], rp.ap()[:, p], g4[:], start=True, stop=True)
                        t2sp = sb_t2s.tile([128, BT], F32R, tag=f"t2s{p}")
                        nc.vector.tensor_mul(t2sp[:], t2[:], eps_[:])
                        t2s.append(t2sp)
                    # ---- U pass + epilogue ----
                    for dt in range(NK):
                        ups = ps_u.tile([128, BT], F32, tag="u")
                        nc.tensor.matmul(
                            ups[:], ut.ap()[:, l, 0, bass.ts(dt, 128)], t2s[0][:],
                            start=True, stop=False,
                        )
                        nc.tensor.matmul(
                            ups[:], ut.ap()[:, l, 1, bass.ts(dt, 128)], t2s[1][:],
                            start=False, stop=True,
                        )
                        # delta + bias (per-partition) -> SBUF via ACT
                        tm2 = sb_tm.tile([128, BT], F32, tag="tm")
                        nc.scalar.activation(
                            tm2[:], ups[:], AF.Identity,
                            bias=bt_sb.ap()[:, l * NK + dt : l * NK + dt + 1],
                        )
                        if l == 0:
                            x0ap = xl[dt].ap()[:, bs].bitcast(F32)
                        else:
                            x0t = sb_x0.tile([128, BT], F32R, tag="x0")
                            nc.sync.dma_start(
                                x0t[:], xT.ap()[bass.ts(dt, 128), bs]
                            )
                            x0ap = x0t[:].bitcast(F32)
                        tm3 = sb_tm.tile([128, BT], F32, tag="tm")
                        nc.vector.tensor_mul(tm3[:], tm2[:], x0ap)
                        adder = nc.gpsimd if ADD_ENGINE == "pool" else nc.vector
                        if l < L - 1:
                            adder.tensor_add(
                                xl[dt].ap()[:, bs],
                                xl[dt].ap()[:, bs].bitcast(F32),
                                tm3[:],
                            )
                        else:
                            ot = sb_ot.tile([128, BT], F32, tag="ot")
                            adder.tensor_add(
                                ot[:], xl[dt].ap()[:, bs].bitcast(F32), tm3[:]
                            )
                            nc.sync.dma_start(
                                OT.ap()[bass.ts(dt, 128), bs], ot[:]
                            )

        if repeat == 1:
            body()
        else:
            with tc.For_i(0, repeat, 1) as _i:
                body(_i)
        if bench:
            dtile = sb_tm.tile([1, 1], F32, tag="dummy")
            nc.sync.dma_start(dtile[:], dummy_in.ap())
            nc.sync.dma_start(dummy_out.ap(), dtile[:])
        ctx.close()

    nc.compile()
    _cache[key] = nc
    return nc


def _prep(x, U, V, C, bias, gates_w):
    """Host-side layout prep. Returns (shared dict, list of per-core dicts)."""
    x = np.ascontiguousarray(x, dtype=np.float32)
    Vt = np.ascontiguousarray(
        V.astype(np.float32).transpose(0, 3, 1, 2).reshape(L, D, E * R)
    )
    Gt = np.ascontiguousarray(gates_w.astype(np.float32).T)
    Cbd = np.zeros((L, 2, 128, 128), dtype=np.float32)
    for l in range(L):
        for p in range(2):
            Cbd[l, p, :R, :R] = C[l, 2 * p].T
            Cbd[l, p, R:, R:] = C[l, 2 * p + 1].T
    Ut = np.zeros((L, 2, 128, D), dtype=np.float32)
    for l in range(L):
        for p in range(2):
            Ut[l, p, :R] = U[l, 2 * p].T
            Ut[l, p, R:] = U[l, 2 * p + 1].T
    RP = np.zeros((3, E, 128), dtype=np.float32)
    for p in range(2):
        for e in range(E):
            if e // 2 == p:
                RP[p, e, (e % 2) * R : (e % 2 + 1) * R] = 1.0
    RP[2] = 1.0
    BTm = np.ascontiguousarray(
        bias.astype(np.float32).reshape(L, NK, 128).transpose(2, 0, 1).reshape(128, L * NK)
    )
    shared = {"Vt": Vt, "Gt": Gt, "Cb": Cbd, "Ut": Ut, "RP": RP, "BT": BTm}
    per_core = []
    for i in range(NCORES):
        xTi = np.ascontiguousarray(x[i * BC : (i + 1) * BC].T)
        per_core.append({"xT": xTi, **shared})
    return per_core


def kernel(x, U, V, C, bias, gates_w):
    nc = _build(1)
    in_maps = _prep(x, U, V, C, bias, gates_w)
    res = run_bass_kernel_spmd(nc, in_maps, list(range(NCORES)))
    out = np.empty((B, D), dtype=np.float32)
    for i in range(NCORES):
        out[i * BC : (i + 1) * BC] = res.results[i]["OT"].T
    return out


if __name__ == "__main__":
    rng = np.random.default_rng(0)
    x = rng.standard_normal((B, D), dtype=np.float32)
    su = (2.0 / (D + R)) ** 0.5
    sc = (2.0 / (R + R)) ** 0.5
    U_ = rng.standard_normal((L, E, D, R), dtype=np.float32) * su
    V_ = rng.standard_normal((L, E, R, D), dtype=np.float32) * su
    C_ = rng.standard_normal((L, E, R, R), dtype=np.float32) * sc
    b_ = np.zeros((L, D), dtype=np.float32)
    g_ = rng.standard_normal((E, D), dtype=np.float32) / np.sqrt(D)
    out = kernel(x, U_, V_, C_, b_, g_)

    # numpy reference
    x0, xl = x, x.astype(np.float64)
    for i in range(L):
        logits = xl @ g_.T.astype(np.float64)
        ex = np.exp(logits - logits.max(axis=1, keepdims=True))
        g = ex / ex.sum(axis=1, keepdims=True)
        t = np.tanh(np.einsum("erd,bd->ber", V_[i].astype(np.float64), xl))
        t = np.tanh(np.einsum("ers,bes->ber", C_[i].astype(np.float64), t))
        t = np.einsum("edr,ber->bed", U_[i].astype(np.float64), t) + b_[i][None, None, :]
        t = x0[:, None, :] * t
        xl = np.einsum("bed,be->bd", t, g) + xl
    err = np.abs(out - xl)
    print(f"absmax={err.max():.4e} rel={err.max()/np.abs(xl).max():.4e}")
